# revision 46
# baseline (speedup 1.0000x reference)
"""AdditiveAttention2D (Bahdanau-style) on 8 Trainium2 NeuronCores.

Reference (per batch b):
    sW = s @ W, hU = h @ U                              [L, D]
    scores[l, m] = sum_d v[d] * tanh(sW[l, d] + hU[m, d])
    attn = softmax_m(scores);  out = attn @ h           [L, D]

Sharding: the B*L = 1024 query rows split across 8 cores (128 rows each,
each core's rows inside one batch). Each core gets its batch's full h
(keys/values) plus replicated W, U, v. No collectives; the host
concatenates the per-core output shards. The host also pre-lays-out the
shards (transposes, bf16 casts, the v-diagonal weight tile) — pure
layout, no FLOPs.

Per-core layout: d lives on partitions. For query q the tanh argument is
hU_T[d, m] + sW_T[d, q] — a per-partition-scalar broadcast add (DVE, 4x
bf16 mode), a bulk fused tanh (ScalarE — the bottleneck engine, ~1
elem/lane/cycle), and the v-weighted d-reduction as a PE matmul with v
embedded in column j of a [D, G] stationary tile so query j lands in
PSUM partition j. Softmax skips max-subtraction (|scores| <= ||v||_1 ~ 9
since |tanh| < 1, so exp cannot overflow fp32). exp-scores go through a
PE transpose to become the stationary operand of the attn @ h
accumulation, then rows are scaled by 1/rowsum.

Emission is software-pipelined so the scalar engine (the bottleneck at
~61us of tanh+exp per core) runs with zero gaps in steady state; chunk
sizes taper at both ends to shorten the ramp-in and the exposed tail.
Measured ~80us end-to-end per NEFF execution at full clock.
"""

from contextlib import ExitStack

import ml_dtypes
import numpy as np

import concourse.bass as bass
import concourse.mybir as mybir
import concourse.tile as tile
from concourse import bacc
from concourse.bass_utils import run_bass_kernel_spmd

F32 = mybir.dt.float32
BF16 = mybir.dt.bfloat16
AF = mybir.ActivationFunctionType

B, L, D = 2, 512, 128
N_CORES = 8
QPC = B * L // N_CORES  # query rows per core (128)
G = 32                  # max queries per chunk (softmax granularity)
MT = L // 128           # 128-row key tiles per batch (4)

# (start query, chunk size, tanh sub-tile sizes). Chunk 0 ramps with small
# tiles; the final chunk is split off and tapered so the closing
# exp/attn/store pipeline after the last tanh is short.
CHUNKS = [
    (0, 32, [2, 2, 4, 8, 16]),
    (32, 32, [16, 16]),
    (64, 32, [16, 16]),
    (96, 16, [8, 4, 4]),
    (112, 8, [8]),
    (120, 8, [4, 2, 2]),
]
NCH = len(CHUNKS)
assert sum(gc for _, gc, _ in CHUNKS) == QPC
assert all(sum(subs) == gc for _, gc, subs in CHUNKS)

# Softmax groups: chunks 0-2 share one 96-row PSUM scores tile (their
# matmuls write partition offsets 0/32/64 — 32-aligned col groups) and a
# single exp: exp cost is per-instruction (FD=512/partition regardless of
# row count), so this removes two exps from the ScalarE critical path and
# two dozen tail instructions. The tapered ending stays split so the
# closing cascade is short.
IDW = 96  # identity width for the widest group's transposes
GROUPS = [(0, 96, [0, 1, 2]), (96, 16, [3]), (112, 8, [4]), (120, 8, [5])]
assert all(
    CHUNKS[cs[0]][0] == q0 and sum(CHUNKS[c][1] for c in cs) == gg
    for q0, gg, cs in GROUPS
)


def build_nc() -> bass.Bass:
    # Bacc (not plain Bass): its compile() runs move_matmul_waits_to_ldweights
    # + generate_event_semaphores, which legalize multi-sem waits down to the
    # 1-wait-per-instruction limit this walrus enforces.
    nc = bacc.Bacc()
    F32R = mybir.dt.float32r
    # Inputs are concatenated on the host so each prologue matmul waits on a
    # single DMA-queue semaphore (fewer event-semaphore hops on the ramp):
    #   hTU  = [hT | U]           bf16 [D, L + D]
    #   WsT  = [W | sT]           f32r [D, D + QPC]
    #   aux  = [vmat | hb | ident] bf16 [128, G*G + L + G]
    hTU_d = nc.declare_dram_parameter("hTU", [D, L + D], BF16, isOutput=False)
    WsT_d = nc.declare_dram_parameter("WsT", [D, D + QPC], F32R, isOutput=False)
    aux_d = nc.declare_dram_parameter(
        "aux", [128, G * G + L + IDW], BF16, isOutput=False
    )
    o_d = nc.declare_dram_parameter("out", [QPC, D], F32, isOutput=True)

    with ExitStack() as ctx:
        tc = ctx.enter_context(tile.TileContext(nc))
        consts = ctx.enter_context(tc.tile_pool(name="consts", bufs=1))
        xpool = ctx.enter_context(tc.tile_pool(name="x", bufs=2))
        tpool = ctx.enter_context(tc.tile_pool(name="t", bufs=2))
        spool = ctx.enter_context(tc.tile_pool(name="small", bufs=3))

        # ---------------- prologue ----------------
        # hTU (the big transfer) gets the sync HWDGE queues to itself; the
        # other loads go via gpsimd's SWDGE queues, whose completion sems
        # are independent of the in-flight hTU (HWDGE queue sems are FIFO,
        # so anything behind hTU on those queues would wait for it).
        hTU_sb = consts.tile([D, L + D], BF16)
        nc.sync.dma_start(out=hTU_sb, in_=hTU_d[:, :])
        hT_sb = hTU_sb[:, 0:L]
        U_sb = hTU_sb[:, L : L + D]
        WsT_sb = consts.tile([D, D + QPC], F32R)
        nc.gpsimd.dma_start(out=WsT_sb, in_=WsT_d[:, :])
        W_sb = WsT_sb[:, 0:D]
        sT_sb = WsT_sb[:, D : D + QPC]
        aux_sb = consts.tile([128, G * G + L + IDW], BF16)
        nc.gpsimd.dma_start(out=aux_sb, in_=aux_d[:, :])
        vmat = aux_sb[:, 0 : G * G].rearrange("p (a b) -> p a b", a=G)
        hb_sb = aux_sb[:, G * G : G * G + L].rearrange("p (t d) -> p t d", t=MT)
        ident = aux_sb[0:IDW, G * G + L : G * G + L + IDW]

        hU_sb = consts.tile([D, L], BF16)
        sW_sb = consts.tile([D, QPC], F32)

        with tc.tile_pool(name="pp_pro", bufs=2, space="PSUM") as pp_pro:
            # sW first: its (small) W/sT loads land while hT is still in
            # flight. Both PSUM->SBUF casts run on the (otherwise idle)
            # scalar engine so the DVE can go straight to the
            # broadcast-adds.
            sW_ps = pp_pro.tile([D, QPC], F32, tag="pro")
            nc.tensor.matmul(sW_ps, W_sb, sT_sb, start=True, stop=True)
            nc.scalar.copy(sW_sb, sW_ps)
            # hU_T[dout, m] = sum_din U[din, dout] * hT[din, m] (bf16 in,
            # fp32 accumulate, cast to bf16 for the adds).
            hU_ps = pp_pro.tile([D, L], F32, tag="pro")
            nc.tensor.matmul(hU_ps, U_sb, hT_sb, start=True, stop=True)
            nc.scalar.copy(hU_sb, hU_ps)

        pp = ctx.enter_context(tc.tile_pool(name="pp", bufs=2, space="PSUM"))

        # ---------------- main loop (software-pipelined emission) --------
        # Per-engine steady-state orders (c = chunk):
        #   ACT: ... tanh(c+1,h0) tanh(c+1,h1) exp(c) ...
        #   DVE: ... adds(c+2) [reduce/recip/eTcopy/scale](c) ...
        #   PE : ... mms(c+1,h0) mms(c+1,h1) [transp/attn](c) ...
        sc_tiles: dict[int, object] = {}   # group idx -> psum scores tile
        exp_tiles: dict[int, object] = {}
        chunk_group = {c: g for g, (_, _, cs) in enumerate(GROUPS) for c in cs}

        def stage_a(c):
            """adds + tanh + v-reduction matmuls for chunk c.

            Chunk 0 ramps with small tanh tiles so the first tanh starts
            as soon as a few broadcast-adds are done; steady-state chunks
            use 16-query tiles (lower per-instruction overhead); the final
            chunk tapers so the closing exp isn't gated by a long matmul
            burst.
            """
            q0, gc, subs = CHUNKS[c]
            g = chunk_group[c]
            gq0, gg, _ = GROUPS[g]
            if g not in sc_tiles:
                sc_group = pp.tile([gg, L], F32, tag="scores")
                sc_tiles[g] = sc_group
            off = q0 - gq0
            sc = sc_tiles[g][off : off + gc, :]
            j0 = 0
            for si, sub in enumerate(subs):
                X = xpool.tile([D, sub, L], BF16, tag=f"X{si % 2}")
                for j in range(j0, j0 + sub):
                    q = q0 + j
                    nc.vector.tensor_scalar_add(
                        X[:, j - j0, :], hU_sb, sW_sb[:, q : q + 1]
                    )
                T = tpool.tile([D, sub, L], BF16, tag=f"T{si % 2}")
                nc.scalar.activation(
                    T.rearrange("p a b -> p (a b)"),
                    X.rearrange("p a b -> p (a b)"),
                    AF.Tanh,
                )
                for j in range(j0, j0 + sub):
                    nc.tensor.matmul(
                        sc,
                        vmat[:, j, 0:gc],
                        T[:, j - j0, :],
                        start=(j == 0),
                        stop=(j == gc - 1),
                    )
                j0 += sub

        sum_tiles: dict[int, object] = {}

        def stage_exp(g):
            """exp for group g. The final group computes its row sums via
            exp's accumulator so its exposed tail skips the DVE
            reduction."""
            gg = GROUPS[g][1]
            exp_sb = spool.tile([gg, L], BF16, tag="exp")
            if g == len(GROUPS) - 1:
                sums = spool.tile([gg, 1], F32, tag="sums")
                nc.scalar.activation(
                    exp_sb, sc_tiles.pop(g), AF.Exp, accum_out=sums
                )
                sum_tiles[g] = sums
            else:
                nc.scalar.activation(exp_sb, sc_tiles.pop(g), AF.Exp)
            exp_tiles[g] = exp_sb

        def stage_tail(g):
            """softmax-normalize + attn @ h + store for group g."""
            q0, gg, _ = GROUPS[g]
            exp_sb = exp_tiles.pop(g)
            if g in sum_tiles:
                sums = sum_tiles.pop(g)
            else:
                sums = spool.tile([gg, 1], F32, tag="sums")
                nc.vector.tensor_reduce(
                    sums, exp_sb, axis=mybir.AxisListType.X, op=mybir.AluOpType.add
                )
            recip = spool.tile([gg, 1], F32, tag="recip")
            nc.vector.reciprocal(recip, sums)
            eT_ps = pp.tile([128, MT, gg], BF16, tag="eT")
            for t in range(MT):
                nc.tensor.transpose(
                    eT_ps[:, t, :],
                    exp_sb[:, t * 128 : (t + 1) * 128],
                    ident[0:gg, 0:gg],
                )
            eT_sb = spool.tile([128, MT, gg], BF16, tag="eTs")
            nc.vector.tensor_copy(eT_sb, eT_ps)
            at_ps = pp.tile([gg, D], F32, tag="attn")
            for t in range(MT):
                nc.tensor.matmul(
                    at_ps,
                    eT_sb[:, t, :],
                    hb_sb[:, t, :],
                    start=(t == 0),
                    stop=(t == MT - 1),
                )
            out_sb = spool.tile([gg, D], F32, tag="out")
            nc.vector.tensor_scalar_mul(out_sb, at_ps, recip[:, 0:1])
            nc.sync.dma_start(out=o_d[q0 : q0 + gg, :], in_=out_sb)

        # ACT order: t0..t3, exp(g0:96q), t4, exp(g1), t5, exp(g2), exp(g3)
        stage_a(0)
        stage_a(1)
        stage_a(2)
        stage_a(3)
        stage_exp(0)
        stage_a(4)
        stage_tail(0)
        stage_exp(1)
        stage_a(5)
        stage_tail(1)
        stage_exp(2)
        stage_tail(2)
        stage_exp(3)
        stage_tail(3)

    nc.compile()
    return nc


_NC_CACHE: list = []


def _get_nc() -> bass.Bass:
    if not _NC_CACHE:
        _NC_CACHE.append(build_nc())
    return _NC_CACHE[0]


def _make_in_maps(s, h, W, U, v):
    s2 = np.ascontiguousarray(np.asarray(s, np.float32).reshape(B * L, D))
    h2 = np.asarray(h, np.float32)
    W2 = np.asarray(W, np.float32)
    U2 = np.asarray(U, np.float32).astype(ml_dtypes.bfloat16)
    v2 = np.asarray(v, np.float32)
    vmat = np.zeros((D, G * G), np.float32)
    for j in range(G):
        vmat[:, j * G + j] = v2[:, 0]
    aux_tail = np.zeros((128, IDW), np.float32)
    aux_tail[:IDW, :] = np.eye(IDW, dtype=np.float32)
    in_maps = []
    for c in range(N_CORES):
        b = c * QPC // L
        h_b = h2[b]  # [L, D]
        hb = h_b.reshape(MT, 128, D).transpose(1, 0, 2).reshape(128, MT * D)
        aux = np.concatenate([vmat, hb, aux_tail], axis=1)
        in_maps.append(
            {
                "hTU": np.ascontiguousarray(
                    np.concatenate(
                        [h_b.T, U2.astype(np.float32)], axis=1
                    ).astype(ml_dtypes.bfloat16)
                ),
                "WsT": np.ascontiguousarray(
                    np.concatenate(
                        [W2, s2[c * QPC : (c + 1) * QPC].T], axis=1
                    )
                ),
                "aux": np.ascontiguousarray(aux.astype(ml_dtypes.bfloat16)),
            }
        )
    return in_maps


def run_spmd(s, h, W, U, v, **kwargs):
    """Run the kernel on 8 cores; returns the BassKernelResults."""
    nc = _get_nc()
    in_maps = _make_in_maps(s, h, W, U, v)
    return run_bass_kernel_spmd(nc, in_maps, core_ids=list(range(N_CORES)), **kwargs)


def kernel(s, h, W, U, v):
    res = run_spmd(s, h, W, U, v)
    shards = [np.asarray(res.results[c]["out"]) for c in range(N_CORES)]
    return np.concatenate(shards, axis=0).reshape(B, L, D).astype(np.float32)


# revision 47
# speedup vs baseline: 1.0217x; 1.0217x over previous
"""AdditiveAttention2D (Bahdanau-style) on 8 Trainium2 NeuronCores.

Reference (per batch b):
    sW = s @ W, hU = h @ U                              [L, D]
    scores[l, m] = sum_d v[d] * tanh(sW[l, d] + hU[m, d])
    attn = softmax_m(scores);  out = attn @ h           [L, D]

Sharding: the B*L = 1024 query rows split across 8 cores (128 rows each,
each core's rows inside one batch). Each core gets its batch's full h
(keys/values) plus replicated W, U, v. No collectives; the host
concatenates the per-core output shards. The host also pre-lays-out the
shards (transposes, bf16 casts, the v-diagonal weight tile) — pure
layout, no FLOPs.

Per-core layout: d lives on partitions. For query q the tanh argument is
hU_T[d, m] + sW_T[d, q] — a per-partition-scalar broadcast add (DVE, 4x
bf16 mode), a bulk fused tanh (ScalarE — the bottleneck engine, ~1
elem/lane/cycle), and the v-weighted d-reduction as a PE matmul with v
embedded in column j of a [D, G] stationary tile so query j lands in
PSUM partition j. Softmax skips max-subtraction (|scores| <= ||v||_1 ~ 9
since |tanh| < 1, so exp cannot overflow fp32). exp-scores go through a
PE transpose to become the stationary operand of the attn @ h
accumulation, then rows are scaled by 1/rowsum.

Emission is software-pipelined so the scalar engine (the bottleneck at
~61us of tanh+exp per core) runs with zero gaps in steady state; chunk
sizes taper at both ends to shorten the ramp-in and the exposed tail.
Measured ~80us end-to-end per NEFF execution at full clock.
"""

from contextlib import ExitStack

import ml_dtypes
import numpy as np

import concourse.bass as bass
import concourse.mybir as mybir
import concourse.tile as tile
from concourse import bacc
from concourse.bass_utils import run_bass_kernel_spmd

F32 = mybir.dt.float32
BF16 = mybir.dt.bfloat16
AF = mybir.ActivationFunctionType

B, L, D = 2, 512, 128
N_CORES = 8
QPC = B * L // N_CORES  # query rows per core (128)
G = 32                  # max queries per chunk (softmax granularity)
MT = L // 128           # 128-row key tiles per batch (4)

# (start query, chunk size, tanh sub-tile sizes). Chunk 0 ramps with small
# tiles; the final chunk is split off and tapered so the closing
# exp/attn/store pipeline after the last tanh is short.
CHUNKS = [
    (0, 32, [4, 4, 8, 16]),
    (32, 32, [16, 16]),
    (64, 32, [16, 16]),
    (96, 16, [8, 4, 4]),
    (112, 8, [8]),
    (120, 8, [4, 2, 2]),
]
NCH = len(CHUNKS)
assert sum(gc for _, gc, _ in CHUNKS) == QPC
assert all(sum(subs) == gc for _, gc, subs in CHUNKS)

# Softmax groups: chunks 0-2 share one 96-row PSUM scores tile (their
# matmuls write partition offsets 0/32/64 — 32-aligned col groups) and a
# single exp: exp cost is per-instruction (FD=512/partition regardless of
# row count), so this removes two exps from the ScalarE critical path and
# two dozen tail instructions. The tapered ending stays split so the
# closing cascade is short.
IDW = 96  # identity width for the widest group's transposes
GROUPS = [(0, 96, [0, 1, 2]), (96, 16, [3]), (112, 8, [4]), (120, 8, [5])]
assert all(
    CHUNKS[cs[0]][0] == q0 and sum(CHUNKS[c][1] for c in cs) == gg
    for q0, gg, cs in GROUPS
)


def build_nc() -> bass.Bass:
    # Bacc (not plain Bass): its compile() runs move_matmul_waits_to_ldweights
    # + generate_event_semaphores, which legalize multi-sem waits down to the
    # 1-wait-per-instruction limit this walrus enforces.
    nc = bacc.Bacc()
    F32R = mybir.dt.float32r
    # Inputs are concatenated on the host so each prologue matmul waits on a
    # single DMA-queue semaphore (fewer event-semaphore hops on the ramp):
    #   hTU  = [hT | U]           bf16 [D, L + D]
    #   WsT  = [W | sT]           f32r [D, D + QPC]
    #   aux  = [vmat | hb | ident] bf16 [128, G*G + L + G]
    hTU_d = nc.declare_dram_parameter("hTU", [D, L + D], BF16, isOutput=False)
    WsT_d = nc.declare_dram_parameter("WsT", [D, D + QPC], F32R, isOutput=False)
    aux_d = nc.declare_dram_parameter(
        "aux", [128, G * G + L + IDW], BF16, isOutput=False
    )
    o_d = nc.declare_dram_parameter("out", [QPC, D], F32, isOutput=True)

    with ExitStack() as ctx:
        tc = ctx.enter_context(tile.TileContext(nc))
        consts = ctx.enter_context(tc.tile_pool(name="consts", bufs=1))
        xpool = ctx.enter_context(tc.tile_pool(name="x", bufs=2))
        tpool = ctx.enter_context(tc.tile_pool(name="t", bufs=2))
        spool = ctx.enter_context(tc.tile_pool(name="small", bufs=3))

        # ---------------- prologue ----------------
        # hTU (the big transfer) gets the sync HWDGE queues to itself; the
        # other loads go via gpsimd's SWDGE queues, whose completion sems
        # are independent of the in-flight hTU (HWDGE queue sems are FIFO,
        # so anything behind hTU on those queues would wait for it).
        hTU_sb = consts.tile([D, L + D], BF16)
        nc.sync.dma_start(out=hTU_sb, in_=hTU_d[:, :])
        hT_sb = hTU_sb[:, 0:L]
        U_sb = hTU_sb[:, L : L + D]
        WsT_sb = consts.tile([D, D + QPC], F32R)
        nc.gpsimd.dma_start(out=WsT_sb, in_=WsT_d[:, :])
        W_sb = WsT_sb[:, 0:D]
        sT_sb = WsT_sb[:, D : D + QPC]
        aux_sb = consts.tile([128, G * G + L + IDW], BF16)
        nc.gpsimd.dma_start(out=aux_sb, in_=aux_d[:, :])
        vmat = aux_sb[:, 0 : G * G].rearrange("p (a b) -> p a b", a=G)
        hb_sb = aux_sb[:, G * G : G * G + L].rearrange("p (t d) -> p t d", t=MT)
        ident = aux_sb[0:IDW, G * G + L : G * G + L + IDW]

        hU_sb = consts.tile([D, L], BF16)
        sW_sb = consts.tile([D, QPC], F32)

        with tc.tile_pool(name="pp_pro", bufs=2, space="PSUM") as pp_pro:
            # sW first: its (small) W/sT loads land while hT is still in
            # flight. Both PSUM->SBUF casts run on the (otherwise idle)
            # scalar engine so the DVE can go straight to the
            # broadcast-adds.
            sW_ps = pp_pro.tile([D, QPC], F32, tag="pro")
            nc.tensor.matmul(sW_ps, W_sb, sT_sb, start=True, stop=True)
            nc.scalar.copy(sW_sb, sW_ps)
            # hU_T[dout, m] = sum_din U[din, dout] * hT[din, m] (bf16 in,
            # fp32 accumulate, cast to bf16 for the adds).
            hU_ps = pp_pro.tile([D, L], F32, tag="pro")
            nc.tensor.matmul(hU_ps, U_sb, hT_sb, start=True, stop=True)
            nc.scalar.copy(hU_sb, hU_ps)

        pp = ctx.enter_context(tc.tile_pool(name="pp", bufs=2, space="PSUM"))

        # ---------------- main loop (software-pipelined emission) --------
        # Per-engine steady-state orders (c = chunk):
        #   ACT: ... tanh(c+1,h0) tanh(c+1,h1) exp(c) ...
        #   DVE: ... adds(c+2) [reduce/recip/eTcopy/scale](c) ...
        #   PE : ... mms(c+1,h0) mms(c+1,h1) [transp/attn](c) ...
        sc_tiles: dict[int, object] = {}   # group idx -> psum scores tile
        exp_tiles: dict[int, object] = {}
        chunk_group = {c: g for g, (_, _, cs) in enumerate(GROUPS) for c in cs}

        def stage_a(c):
            """adds + tanh + v-reduction matmuls for chunk c.

            Chunk 0 ramps with small tanh tiles so the first tanh starts
            as soon as a few broadcast-adds are done; steady-state chunks
            use 16-query tiles (lower per-instruction overhead); the final
            chunk tapers so the closing exp isn't gated by a long matmul
            burst.
            """
            q0, gc, subs = CHUNKS[c]
            g = chunk_group[c]
            gq0, gg, _ = GROUPS[g]
            if g not in sc_tiles:
                sc_group = pp.tile([gg, L], F32, tag="scores")
                sc_tiles[g] = sc_group
            off = q0 - gq0
            sc = sc_tiles[g][off : off + gc, :]
            j0 = 0
            for si, sub in enumerate(subs):
                X = xpool.tile([D, sub, L], BF16, tag=f"X{si % 2}")
                for j in range(j0, j0 + sub):
                    q = q0 + j
                    nc.vector.tensor_scalar_add(
                        X[:, j - j0, :], hU_sb, sW_sb[:, q : q + 1]
                    )
                T = tpool.tile([D, sub, L], BF16, tag=f"T{si % 2}")
                nc.scalar.activation(
                    T.rearrange("p a b -> p (a b)"),
                    X.rearrange("p a b -> p (a b)"),
                    AF.Tanh,
                )
                for j in range(j0, j0 + sub):
                    nc.tensor.matmul(
                        sc,
                        vmat[:, j, 0:gc],
                        T[:, j - j0, :],
                        start=(j == 0),
                        stop=(j == gc - 1),
                    )
                j0 += sub

        sum_tiles: dict[int, object] = {}

        def stage_exp(g):
            """exp for group g. The final group computes its row sums via
            exp's accumulator so its exposed tail skips the DVE
            reduction."""
            gg = GROUPS[g][1]
            exp_sb = spool.tile([gg, L], BF16, tag="exp")
            if g == len(GROUPS) - 1:
                sums = spool.tile([gg, 1], F32, tag="sums")
                nc.scalar.activation(
                    exp_sb, sc_tiles.pop(g), AF.Exp, accum_out=sums
                )
                sum_tiles[g] = sums
            else:
                nc.scalar.activation(exp_sb, sc_tiles.pop(g), AF.Exp)
            exp_tiles[g] = exp_sb

        def stage_tail(g):
            """softmax-normalize + attn @ h + store for group g."""
            q0, gg, _ = GROUPS[g]
            exp_sb = exp_tiles.pop(g)
            if g in sum_tiles:
                sums = sum_tiles.pop(g)
            else:
                sums = spool.tile([gg, 1], F32, tag="sums")
                nc.vector.tensor_reduce(
                    sums, exp_sb, axis=mybir.AxisListType.X, op=mybir.AluOpType.add
                )
            recip = spool.tile([gg, 1], F32, tag="recip")
            nc.vector.reciprocal(recip, sums)
            eT_ps = pp.tile([128, MT, gg], BF16, tag="eT")
            for t in range(MT):
                nc.tensor.transpose(
                    eT_ps[:, t, :],
                    exp_sb[:, t * 128 : (t + 1) * 128],
                    ident[0:gg, 0:gg],
                )
            eT_sb = spool.tile([128, MT, gg], BF16, tag="eTs")
            nc.vector.tensor_copy(eT_sb, eT_ps)
            at_ps = pp.tile([gg, D], F32, tag="attn")
            for t in range(MT):
                nc.tensor.matmul(
                    at_ps,
                    eT_sb[:, t, :],
                    hb_sb[:, t, :],
                    start=(t == 0),
                    stop=(t == MT - 1),
                )
            out_sb = spool.tile([gg, D], F32, tag="out")
            nc.vector.tensor_scalar_mul(out_sb, at_ps, recip[:, 0:1])
            nc.sync.dma_start(out=o_d[q0 : q0 + gg, :], in_=out_sb)

        # ACT order: t0..t3, exp(g0:96q), t4, exp(g1), t5, exp(g2), exp(g3)
        stage_a(0)
        stage_a(1)
        stage_a(2)
        stage_a(3)
        stage_exp(0)
        stage_a(4)
        stage_tail(0)
        stage_exp(1)
        stage_a(5)
        stage_tail(1)
        stage_exp(2)
        stage_tail(2)
        stage_exp(3)
        stage_tail(3)

    nc.compile()
    return nc


_NC_CACHE: list = []


def _get_nc() -> bass.Bass:
    if not _NC_CACHE:
        _NC_CACHE.append(build_nc())
    return _NC_CACHE[0]


def _make_in_maps(s, h, W, U, v):
    s2 = np.ascontiguousarray(np.asarray(s, np.float32).reshape(B * L, D))
    h2 = np.asarray(h, np.float32)
    W2 = np.asarray(W, np.float32)
    U2 = np.asarray(U, np.float32).astype(ml_dtypes.bfloat16)
    v2 = np.asarray(v, np.float32)
    vmat = np.zeros((D, G * G), np.float32)
    for j in range(G):
        vmat[:, j * G + j] = v2[:, 0]
    aux_tail = np.zeros((128, IDW), np.float32)
    aux_tail[:IDW, :] = np.eye(IDW, dtype=np.float32)
    in_maps = []
    for c in range(N_CORES):
        b = c * QPC // L
        h_b = h2[b]  # [L, D]
        hb = h_b.reshape(MT, 128, D).transpose(1, 0, 2).reshape(128, MT * D)
        aux = np.concatenate([vmat, hb, aux_tail], axis=1)
        in_maps.append(
            {
                "hTU": np.ascontiguousarray(
                    np.concatenate(
                        [h_b.T, U2.astype(np.float32)], axis=1
                    ).astype(ml_dtypes.bfloat16)
                ),
                "WsT": np.ascontiguousarray(
                    np.concatenate(
                        [W2, s2[c * QPC : (c + 1) * QPC].T], axis=1
                    )
                ),
                "aux": np.ascontiguousarray(aux.astype(ml_dtypes.bfloat16)),
            }
        )
    return in_maps


def run_spmd(s, h, W, U, v, **kwargs):
    """Run the kernel on 8 cores; returns the BassKernelResults."""
    nc = _get_nc()
    in_maps = _make_in_maps(s, h, W, U, v)
    return run_bass_kernel_spmd(nc, in_maps, core_ids=list(range(N_CORES)), **kwargs)


def kernel(s, h, W, U, v):
    res = run_spmd(s, h, W, U, v)
    shards = [np.asarray(res.results[c]["out"]) for c in range(N_CORES)]
    return np.concatenate(shards, axis=0).reshape(B, L, D).astype(np.float32)


# revision 49
# speedup vs baseline: 1.0409x; 1.0187x over previous
"""AdditiveAttention2D (Bahdanau-style) on 8 Trainium2 NeuronCores.

Reference (per batch b):
    sW = s @ W, hU = h @ U                              [L, D]
    scores[l, m] = sum_d v[d] * tanh(sW[l, d] + hU[m, d])
    attn = softmax_m(scores);  out = attn @ h           [L, D]

Sharding: the B*L = 1024 query rows split across 8 cores (128 rows each,
each core's rows inside one batch). Each core gets its batch's full h
(keys/values) plus replicated W, U, v. No collectives; the host
concatenates the per-core output shards. The host also pre-lays-out the
shards (transposes, bf16 casts, the v-diagonal weight tile) — pure
layout, no FLOPs.

Per-core layout: d lives on partitions. For query q the tanh argument is
hU_T[d, m] + sW_T[d, q] — a per-partition-scalar broadcast add (DVE, 4x
bf16 mode), a bulk fused tanh (ScalarE — the bottleneck engine, ~1
elem/lane/cycle), and the v-weighted d-reduction as a PE matmul with v
embedded in column j of a [D, G] stationary tile so query j lands in
PSUM partition j. Softmax skips max-subtraction (|scores| <= ||v||_1 ~ 9
since |tanh| < 1, so exp cannot overflow fp32). exp-scores go through a
PE transpose to become the stationary operand of the attn @ h
accumulation, then rows are scaled by 1/rowsum.

Emission is software-pipelined so the scalar engine (the bottleneck at
~61us of tanh+exp per core) runs with zero gaps in steady state; chunk
sizes taper at both ends to shorten the ramp-in and the exposed tail.
Measured ~80us end-to-end per NEFF execution at full clock.
"""

from contextlib import ExitStack

import ml_dtypes
import numpy as np

import concourse.bass as bass
import concourse.mybir as mybir
import concourse.tile as tile
from concourse import bacc
from concourse.bass_utils import run_bass_kernel_spmd

F32 = mybir.dt.float32
BF16 = mybir.dt.bfloat16
AF = mybir.ActivationFunctionType

B, L, D = 2, 512, 128
N_CORES = 8
QPC = B * L // N_CORES  # query rows per core (128)
G = 32                  # max queries per chunk (softmax granularity)
MT = L // 128           # 128-row key tiles per batch (4)

# (start query, chunk size, tanh sub-tile sizes). Chunk 0 ramps with small
# tiles; the final chunk is split off and tapered so the closing
# exp/attn/store pipeline after the last tanh is short.
CHUNKS = [
    (0, 32, [4, 4, 8, 16]),
    (32, 32, [16, 16]),
    (64, 32, [16, 16]),
    (96, 16, [8, 4, 4]),
    (112, 8, [8]),
    (120, 8, [4, 2, 2]),
]
NCH = len(CHUNKS)
assert sum(gc for _, gc, _ in CHUNKS) == QPC
assert all(sum(subs) == gc for _, gc, subs in CHUNKS)

# Softmax groups: chunks 0-2 share one 96-row PSUM scores tile (their
# matmuls write partition offsets 0/32/64 — 32-aligned col groups) and a
# single exp: exp cost is per-instruction (FD=512/partition regardless of
# row count), so this removes two exps from the ScalarE critical path and
# two dozen tail instructions. The tapered ending stays split so the
# closing cascade is short.
IDW = 96  # identity width for the widest group's transposes
GROUPS = [(0, 96, [0, 1, 2]), (96, 16, [3]), (112, 8, [4]), (120, 8, [5])]
assert all(
    CHUNKS[cs[0]][0] == q0 and sum(CHUNKS[c][1] for c in cs) == gg
    for q0, gg, cs in GROUPS
)


def build_nc() -> bass.Bass:
    # Bacc (not plain Bass): its compile() runs move_matmul_waits_to_ldweights
    # + generate_event_semaphores, which legalize multi-sem waits down to the
    # 1-wait-per-instruction limit this walrus enforces.
    nc = bacc.Bacc()
    F32R = mybir.dt.float32r
    # Inputs are concatenated on the host so each prologue matmul waits on a
    # single DMA-queue semaphore (fewer event-semaphore hops on the ramp):
    #   hTU  = [hT | U]           bf16 [D, L + D]
    #   WsT  = [W | sT]           f32r [D, D + QPC]
    #   aux  = [vmat | hb | ident] bf16 [128, G*G + L + G]
    hTU_d = nc.declare_dram_parameter("hTU", [D, L + D], BF16, isOutput=False)
    WsT_d = nc.declare_dram_parameter("WsT", [D, D + QPC], F32R, isOutput=False)
    aux_d = nc.declare_dram_parameter(
        "aux", [128, G * G + L + IDW + 2], BF16, isOutput=False
    )
    o_d = nc.declare_dram_parameter("out", [QPC, D], F32, isOutput=True)

    with ExitStack() as ctx:
        tc = ctx.enter_context(tile.TileContext(nc))
        consts = ctx.enter_context(tc.tile_pool(name="consts", bufs=1))
        xpool = ctx.enter_context(tc.tile_pool(name="x", bufs=2))
        tpool = ctx.enter_context(tc.tile_pool(name="t", bufs=2))
        spool = ctx.enter_context(tc.tile_pool(name="small", bufs=3))

        # ---------------- prologue ----------------
        # hTU (the big transfer) gets the sync HWDGE queues to itself; the
        # other loads go via gpsimd's SWDGE queues, whose completion sems
        # are independent of the in-flight hTU (HWDGE queue sems are FIFO,
        # so anything behind hTU on those queues would wait for it).
        hTU_sb = consts.tile([D, L + D], BF16)
        nc.sync.dma_start(out=hTU_sb, in_=hTU_d[:, :])
        hT_sb = hTU_sb[:, 0:L]
        U_sb = hTU_sb[:, L : L + D]
        WsT_sb = consts.tile([D, D + QPC], F32R)
        nc.gpsimd.dma_start(out=WsT_sb, in_=WsT_d[:, :])
        W_sb = WsT_sb[:, 0:D]
        sT_sb = WsT_sb[:, D : D + QPC]
        aux_sb = consts.tile([128, G * G + L + IDW + 2], BF16)
        nc.gpsimd.dma_start(out=aux_sb, in_=aux_d[:, :])
        vmat = aux_sb[:, 0 : G * G].rearrange("p (a b) -> p a b", a=G)
        hb_sb = aux_sb[:, G * G : G * G + L].rearrange("p (t d) -> p t d", t=MT)
        ident = aux_sb[0:IDW, G * G + L : G * G + L + IDW]
        # all-zero column (host-provided) used as the explicit activation
        # bias, replacing the Bass const-AP pool
        zbias = aux_sb[:, G * G + L + IDW : G * G + L + IDW + 1]

        hU_sb = consts.tile([D, L], BF16)
        sW_sb = consts.tile([D, QPC], F32)

        with tc.tile_pool(name="pp_pro", bufs=2, space="PSUM") as pp_pro:
            # sW first: its (small) W/sT loads land while hT is still in
            # flight. Both PSUM->SBUF casts run on the (otherwise idle)
            # scalar engine so the DVE can go straight to the
            # broadcast-adds.
            sW_ps = pp_pro.tile([D, QPC], F32, tag="pro")
            nc.tensor.matmul(sW_ps, W_sb, sT_sb, start=True, stop=True)
            nc.scalar.copy(sW_sb, sW_ps)
            # hU_T[dout, m] = sum_din U[din, dout] * hT[din, m] (bf16 in,
            # fp32 accumulate, cast to bf16 for the adds).
            hU_ps = pp_pro.tile([D, L], F32, tag="pro")
            nc.tensor.matmul(hU_ps, U_sb, hT_sb, start=True, stop=True)
            nc.scalar.copy(hU_sb, hU_ps)

        pp = ctx.enter_context(tc.tile_pool(name="pp", bufs=2, space="PSUM"))

        # ---------------- main loop (software-pipelined emission) --------
        # Per-engine steady-state orders (c = chunk):
        #   ACT: ... tanh(c+1,h0) tanh(c+1,h1) exp(c) ...
        #   DVE: ... adds(c+2) [reduce/recip/eTcopy/scale](c) ...
        #   PE : ... mms(c+1,h0) mms(c+1,h1) [transp/attn](c) ...
        sc_tiles: dict[int, object] = {}   # group idx -> psum scores tile
        exp_tiles: dict[int, object] = {}
        chunk_group = {c: g for g, (_, _, cs) in enumerate(GROUPS) for c in cs}

        def stage_a(c):
            """adds + tanh + v-reduction matmuls for chunk c.

            Chunk 0 ramps with small tanh tiles so the first tanh starts
            as soon as a few broadcast-adds are done; steady-state chunks
            use 16-query tiles (lower per-instruction overhead); the final
            chunk tapers so the closing exp isn't gated by a long matmul
            burst.
            """
            q0, gc, subs = CHUNKS[c]
            g = chunk_group[c]
            gq0, gg, _ = GROUPS[g]
            if g not in sc_tiles:
                sc_group = pp.tile([gg, L], F32, tag="scores")
                sc_tiles[g] = sc_group
            off = q0 - gq0
            sc = sc_tiles[g][off : off + gc, :]
            j0 = 0
            for si, sub in enumerate(subs):
                X = xpool.tile([D, sub, L], BF16, tag=f"X{si % 2}")
                for j in range(j0, j0 + sub):
                    q = q0 + j
                    nc.vector.tensor_scalar_add(
                        X[:, j - j0, :], hU_sb, sW_sb[:, q : q + 1]
                    )
                T = tpool.tile([D, sub, L], BF16, tag=f"T{si % 2}")
                nc.scalar.activation(
                    T.rearrange("p a b -> p (a b)"),
                    X.rearrange("p a b -> p (a b)"),
                    AF.Tanh,
                    bias=zbias[0:D, :],
                )
                for j in range(j0, j0 + sub):
                    nc.tensor.matmul(
                        sc,
                        vmat[:, j, 0:gc],
                        T[:, j - j0, :],
                        start=(j == 0),
                        stop=(j == gc - 1),
                    )
                j0 += sub

        sum_tiles: dict[int, object] = {}

        def stage_exp(g):
            """exp for group g. The final group computes its row sums via
            exp's accumulator so its exposed tail skips the DVE
            reduction."""
            gg = GROUPS[g][1]
            exp_sb = spool.tile([gg, L], BF16, tag="exp")
            if g == len(GROUPS) - 1:
                sums = spool.tile([gg, 1], F32, tag="sums")
                nc.scalar.activation(
                    exp_sb, sc_tiles.pop(g), AF.Exp,
                    bias=zbias[0:gg, :], accum_out=sums,
                )
                sum_tiles[g] = sums
            else:
                nc.scalar.activation(
                    exp_sb, sc_tiles.pop(g), AF.Exp, bias=zbias[0:gg, :]
                )
            exp_tiles[g] = exp_sb

        def stage_tail(g):
            """softmax-normalize + attn @ h + store for group g."""
            q0, gg, _ = GROUPS[g]
            exp_sb = exp_tiles.pop(g)
            if g in sum_tiles:
                sums = sum_tiles.pop(g)
            else:
                sums = spool.tile([gg, 1], F32, tag="sums")
                nc.vector.tensor_reduce(
                    sums, exp_sb, axis=mybir.AxisListType.X, op=mybir.AluOpType.add
                )
            recip = spool.tile([gg, 1], F32, tag="recip")
            nc.vector.reciprocal(recip, sums)
            eT_ps = pp.tile([128, MT, gg], BF16, tag="eT")
            for t in range(MT):
                nc.tensor.transpose(
                    eT_ps[:, t, :],
                    exp_sb[:, t * 128 : (t + 1) * 128],
                    ident[0:gg, 0:gg],
                )
            eT_sb = spool.tile([128, MT, gg], BF16, tag="eTs")
            nc.vector.tensor_copy(eT_sb, eT_ps)
            at_ps = pp.tile([gg, D], F32, tag="attn")
            for t in range(MT):
                nc.tensor.matmul(
                    at_ps,
                    eT_sb[:, t, :],
                    hb_sb[:, t, :],
                    start=(t == 0),
                    stop=(t == MT - 1),
                )
            out_sb = spool.tile([gg, D], F32, tag="out")
            nc.vector.tensor_scalar_mul(out_sb, at_ps, recip[:, 0:1])
            nc.sync.dma_start(out=o_d[q0 : q0 + gg, :], in_=out_sb)

        # ACT order: t0..t3, exp(g0:96q), t4, exp(g1), t5, exp(g2), exp(g3)
        stage_a(0)
        stage_a(1)
        stage_a(2)
        stage_a(3)
        stage_exp(0)
        stage_a(4)
        stage_tail(0)
        stage_exp(1)
        stage_a(5)
        stage_tail(1)
        stage_exp(2)
        stage_tail(2)
        stage_exp(3)
        stage_tail(3)

    # Nothing reads the Bass const-AP pool now (explicit zbias instead), so
    # drop its preamble memsets — they would run first on gpsimd, delaying
    # the DMA issues and anchoring neuron-profile's first_useful_time.
    for bb in nc.main_func.blocks:
        dead = [
            i
            for i in bb.instructions
            if i.opcode == "Memset"
            and i.outs
            and str(getattr(i.outs[0], "memref", "")).startswith("const-")
        ]
        for i in dead:
            bb.instructions.remove(i)

    nc.compile()
    return nc


_NC_CACHE: list = []


def _get_nc() -> bass.Bass:
    if not _NC_CACHE:
        _NC_CACHE.append(build_nc())
    return _NC_CACHE[0]


def _make_in_maps(s, h, W, U, v):
    s2 = np.ascontiguousarray(np.asarray(s, np.float32).reshape(B * L, D))
    h2 = np.asarray(h, np.float32)
    W2 = np.asarray(W, np.float32)
    U2 = np.asarray(U, np.float32).astype(ml_dtypes.bfloat16)
    v2 = np.asarray(v, np.float32)
    vmat = np.zeros((D, G * G), np.float32)
    for j in range(G):
        vmat[:, j * G + j] = v2[:, 0]
    aux_tail = np.zeros((128, IDW + 2), np.float32)
    aux_tail[:IDW, :IDW] = np.eye(IDW, dtype=np.float32)
    in_maps = []
    for c in range(N_CORES):
        b = c * QPC // L
        h_b = h2[b]  # [L, D]
        hb = h_b.reshape(MT, 128, D).transpose(1, 0, 2).reshape(128, MT * D)
        aux = np.concatenate([vmat, hb, aux_tail], axis=1)
        in_maps.append(
            {
                "hTU": np.ascontiguousarray(
                    np.concatenate(
                        [h_b.T, U2.astype(np.float32)], axis=1
                    ).astype(ml_dtypes.bfloat16)
                ),
                "WsT": np.ascontiguousarray(
                    np.concatenate(
                        [W2, s2[c * QPC : (c + 1) * QPC].T], axis=1
                    )
                ),
                "aux": np.ascontiguousarray(aux.astype(ml_dtypes.bfloat16)),
            }
        )
    return in_maps


def run_spmd(s, h, W, U, v, **kwargs):
    """Run the kernel on 8 cores; returns the BassKernelResults."""
    nc = _get_nc()
    in_maps = _make_in_maps(s, h, W, U, v)
    return run_bass_kernel_spmd(nc, in_maps, core_ids=list(range(N_CORES)), **kwargs)


def kernel(s, h, W, U, v):
    res = run_spmd(s, h, W, U, v)
    shards = [np.asarray(res.results[c]["out"]) for c in range(N_CORES)]
    return np.concatenate(shards, axis=0).reshape(B, L, D).astype(np.float32)


# revision 50
# speedup vs baseline: 1.0853x; 1.0427x over previous
"""AdditiveAttention2D (Bahdanau-style) on 8 Trainium2 NeuronCores.

Reference (per batch b):
    sW = s @ W, hU = h @ U                              [L, D]
    scores[l, m] = sum_d v[d] * tanh(sW[l, d] + hU[m, d])
    attn = softmax_m(scores);  out = attn @ h           [L, D]

Sharding: the B*L = 1024 query rows split across 8 cores (128 rows each,
each core's rows inside one batch). Each core gets its batch's full h
(keys/values) plus replicated W, U, v. No collectives; the host
concatenates the per-core output shards. The host also pre-lays-out the
shards (transposes, bf16 casts, the v-diagonal weight tile) — pure
layout, no FLOPs.

Per-core layout: d lives on partitions. For query q the tanh argument is
hU_T[d, m] + sW_T[d, q] — a per-partition-scalar broadcast add (DVE, 4x
bf16 mode), a bulk fused tanh (ScalarE — the bottleneck engine, ~1
elem/lane/cycle), and the v-weighted d-reduction as a PE matmul with v
embedded in column j of a [D, G] stationary tile so query j lands in
PSUM partition j. Softmax skips max-subtraction (|scores| <= ||v||_1 ~ 9
since |tanh| < 1, so exp cannot overflow fp32). exp-scores go through a
PE transpose to become the stationary operand of the attn @ h
accumulation, then rows are scaled by 1/rowsum.

Emission is software-pipelined so the scalar engine (the bottleneck at
~61us of tanh+exp per core) runs with zero gaps in steady state; chunk
sizes taper at both ends to shorten the ramp-in and the exposed tail.
Measured ~80us end-to-end per NEFF execution at full clock.
"""

from contextlib import ExitStack

import ml_dtypes
import numpy as np

import concourse.bass as bass
import concourse.mybir as mybir
import concourse.tile as tile
from concourse import bacc
from concourse.bass_utils import run_bass_kernel_spmd

F32 = mybir.dt.float32
BF16 = mybir.dt.bfloat16
AF = mybir.ActivationFunctionType

B, L, D = 2, 512, 128
N_CORES = 8
QPC = B * L // N_CORES  # query rows per core (128)
G = 32                  # max queries per chunk (softmax granularity)
MT = L // 128           # 128-row key tiles per batch (4)

# (start query, chunk size, tanh sub-tile sizes). Chunk 0 ramps with small
# tiles; the final chunk is split off and tapered so the closing
# exp/attn/store pipeline after the last tanh is short.
CHUNKS = [
    (0, 32, [4, 4, 8, 16]),
    (32, 32, [16, 16]),
    (64, 32, [16, 16]),
    (96, 16, [8, 4, 4]),
    (112, 8, [8]),
    (120, 8, [4, 2, 2]),
]
NCH = len(CHUNKS)
assert sum(gc for _, gc, _ in CHUNKS) == QPC
assert all(sum(subs) == gc for _, gc, subs in CHUNKS)

# Softmax groups: chunks 0-2 share one 96-row PSUM scores tile (their
# matmuls write partition offsets 0/32/64 — 32-aligned col groups) and a
# single exp: exp cost is per-instruction (FD=512/partition regardless of
# row count), so this removes two exps from the ScalarE critical path and
# two dozen tail instructions. The tapered ending stays split so the
# closing cascade is short.
IDW = 96  # identity width for the widest group's transposes
GROUPS = [(0, 96, [0, 1, 2]), (96, 16, [3]), (112, 8, [4]), (120, 8, [5])]
assert all(
    CHUNKS[cs[0]][0] == q0 and sum(CHUNKS[c][1] for c in cs) == gg
    for q0, gg, cs in GROUPS
)


def build_nc() -> bass.Bass:
    # Bacc (not plain Bass): its compile() runs move_matmul_waits_to_ldweights
    # + generate_event_semaphores, which legalize multi-sem waits down to the
    # 1-wait-per-instruction limit this walrus enforces.
    nc = bacc.Bacc()
    F32R = mybir.dt.float32r
    # Inputs are concatenated on the host so each prologue matmul waits on a
    # single DMA-queue semaphore (fewer event-semaphore hops on the ramp):
    #   hTU  = [hT | U]           bf16 [D, L + D]
    #   WsT  = [W | sT]           f32r [D, D + QPC]
    #   aux  = [vmat | hb | ident] bf16 [128, G*G + L + G]
    hTU_d = nc.declare_dram_parameter("hTU", [D, L + D], BF16, isOutput=False)
    WsT_d = nc.declare_dram_parameter("WsT", [D, D + QPC], F32R, isOutput=False)
    aux_d = nc.declare_dram_parameter(
        "aux", [128, G * G + L + IDW + 2], BF16, isOutput=False
    )
    o_d = nc.declare_dram_parameter("out", [QPC, D], F32, isOutput=True)

    with ExitStack() as ctx:
        tc = ctx.enter_context(tile.TileContext(nc))
        consts = ctx.enter_context(tc.tile_pool(name="consts", bufs=1))
        xpool = ctx.enter_context(tc.tile_pool(name="x", bufs=2))
        tpool = ctx.enter_context(tc.tile_pool(name="t", bufs=2))
        spool = ctx.enter_context(tc.tile_pool(name="small", bufs=3))

        # ---------------- prologue ----------------
        # hTU (the big transfer) gets the sync HWDGE queues to itself; the
        # other loads go via gpsimd's SWDGE queues, whose completion sems
        # are independent of the in-flight hTU (HWDGE queue sems are FIFO,
        # so anything behind hTU on those queues would wait for it).
        hTU_sb = consts.tile([D, L + D], BF16)
        nc.sync.dma_start(out=hTU_sb, in_=hTU_d[:, :])
        hT_sb = hTU_sb[:, 0:L]
        U_sb = hTU_sb[:, L : L + D]
        WsT_sb = consts.tile([D, D + QPC], F32R)
        nc.scalar.dma_start(out=WsT_sb, in_=WsT_d[:, :])
        W_sb = WsT_sb[:, 0:D]
        sT_sb = WsT_sb[:, D : D + QPC]
        aux_sb = consts.tile([128, G * G + L + IDW + 2], BF16)
        nc.scalar.dma_start(out=aux_sb, in_=aux_d[:, :])
        vmat = aux_sb[:, 0 : G * G].rearrange("p (a b) -> p a b", a=G)
        hb_sb = aux_sb[:, G * G : G * G + L].rearrange("p (t d) -> p t d", t=MT)
        ident = aux_sb[0:IDW, G * G + L : G * G + L + IDW]
        # all-zero column (host-provided) used as the explicit activation
        # bias, replacing the Bass const-AP pool
        zbias = aux_sb[:, G * G + L + IDW : G * G + L + IDW + 1]

        hU_sb = consts.tile([D, L], BF16)
        sW_sb = consts.tile([D, QPC], F32)

        with tc.tile_pool(name="pp_pro", bufs=2, space="PSUM") as pp_pro:
            # sW first: its (small) W/sT loads land while hT is still in
            # flight. Both PSUM->SBUF casts run on the (otherwise idle)
            # scalar engine so the DVE can go straight to the
            # broadcast-adds.
            sW_ps = pp_pro.tile([D, QPC], F32, tag="pro")
            nc.tensor.matmul(sW_ps, W_sb, sT_sb, start=True, stop=True)
            nc.scalar.copy(sW_sb, sW_ps)
            # hU_T[dout, m] = sum_din U[din, dout] * hT[din, m] (bf16 in,
            # fp32 accumulate, cast to bf16 for the adds).
            hU_ps = pp_pro.tile([D, L], F32, tag="pro")
            nc.tensor.matmul(hU_ps, U_sb, hT_sb, start=True, stop=True)
            nc.scalar.copy(hU_sb, hU_ps)

        pp = ctx.enter_context(tc.tile_pool(name="pp", bufs=2, space="PSUM"))

        # ---------------- main loop (software-pipelined emission) --------
        # Per-engine steady-state orders (c = chunk):
        #   ACT: ... tanh(c+1,h0) tanh(c+1,h1) exp(c) ...
        #   DVE: ... adds(c+2) [reduce/recip/eTcopy/scale](c) ...
        #   PE : ... mms(c+1,h0) mms(c+1,h1) [transp/attn](c) ...
        sc_tiles: dict[int, object] = {}   # group idx -> psum scores tile
        exp_tiles: dict[int, object] = {}
        chunk_group = {c: g for g, (_, _, cs) in enumerate(GROUPS) for c in cs}

        def stage_a(c):
            """adds + tanh + v-reduction matmuls for chunk c.

            Chunk 0 ramps with small tanh tiles so the first tanh starts
            as soon as a few broadcast-adds are done; steady-state chunks
            use 16-query tiles (lower per-instruction overhead); the final
            chunk tapers so the closing exp isn't gated by a long matmul
            burst.
            """
            q0, gc, subs = CHUNKS[c]
            g = chunk_group[c]
            gq0, gg, _ = GROUPS[g]
            if g not in sc_tiles:
                sc_group = pp.tile([gg, L], F32, tag="scores")
                sc_tiles[g] = sc_group
            off = q0 - gq0
            sc = sc_tiles[g][off : off + gc, :]
            j0 = 0
            for si, sub in enumerate(subs):
                X = xpool.tile([D, sub, L], BF16, tag=f"X{si % 2}")
                for j in range(j0, j0 + sub):
                    q = q0 + j
                    nc.vector.tensor_scalar_add(
                        X[:, j - j0, :], hU_sb, sW_sb[:, q : q + 1]
                    )
                T = tpool.tile([D, sub, L], BF16, tag=f"T{si % 2}")
                nc.scalar.activation(
                    T.rearrange("p a b -> p (a b)"),
                    X.rearrange("p a b -> p (a b)"),
                    AF.Tanh,
                    bias=zbias[0:D, :],
                )
                for j in range(j0, j0 + sub):
                    nc.tensor.matmul(
                        sc,
                        vmat[:, j, 0:gc],
                        T[:, j - j0, :],
                        start=(j == 0),
                        stop=(j == gc - 1),
                    )
                j0 += sub

        sum_tiles: dict[int, object] = {}

        def stage_exp(g):
            """exp for group g. The final group computes its row sums via
            exp's accumulator so its exposed tail skips the DVE
            reduction."""
            gg = GROUPS[g][1]
            exp_sb = spool.tile([gg, L], BF16, tag="exp")
            if g == len(GROUPS) - 1:
                sums = spool.tile([gg, 1], F32, tag="sums")
                nc.scalar.activation(
                    exp_sb, sc_tiles.pop(g), AF.Exp,
                    bias=zbias[0:gg, :], accum_out=sums,
                )
                sum_tiles[g] = sums
            else:
                nc.scalar.activation(
                    exp_sb, sc_tiles.pop(g), AF.Exp, bias=zbias[0:gg, :]
                )
            exp_tiles[g] = exp_sb

        def stage_tail(g):
            """softmax-normalize + attn @ h + store for group g."""
            q0, gg, _ = GROUPS[g]
            exp_sb = exp_tiles.pop(g)
            if g in sum_tiles:
                sums = sum_tiles.pop(g)
            else:
                sums = spool.tile([gg, 1], F32, tag="sums")
                nc.vector.tensor_reduce(
                    sums, exp_sb, axis=mybir.AxisListType.X, op=mybir.AluOpType.add
                )
            recip = spool.tile([gg, 1], F32, tag="recip")
            nc.vector.reciprocal(recip, sums)
            eT_ps = pp.tile([128, MT, gg], BF16, tag="eT")
            for t in range(MT):
                nc.tensor.transpose(
                    eT_ps[:, t, :],
                    exp_sb[:, t * 128 : (t + 1) * 128],
                    ident[0:gg, 0:gg],
                )
            eT_sb = spool.tile([128, MT, gg], BF16, tag="eTs")
            nc.vector.tensor_copy(eT_sb, eT_ps)
            at_ps = pp.tile([gg, D], F32, tag="attn")
            for t in range(MT):
                nc.tensor.matmul(
                    at_ps,
                    eT_sb[:, t, :],
                    hb_sb[:, t, :],
                    start=(t == 0),
                    stop=(t == MT - 1),
                )
            out_sb = spool.tile([gg, D], F32, tag="out")
            nc.vector.tensor_scalar_mul(out_sb, at_ps, recip[:, 0:1])
            nc.sync.dma_start(out=o_d[q0 : q0 + gg, :], in_=out_sb)

        # ACT order: t0..t3, exp(g0:96q), t4, exp(g1), t5, exp(g2), exp(g3)
        stage_a(0)
        stage_a(1)
        stage_a(2)
        stage_a(3)
        stage_exp(0)
        stage_a(4)
        stage_tail(0)
        stage_exp(1)
        stage_a(5)
        stage_tail(1)
        stage_exp(2)
        stage_tail(2)
        stage_exp(3)
        stage_tail(3)

    # Nothing reads the Bass const-AP pool now (explicit zbias instead), so
    # drop its preamble memsets — they would run first on gpsimd, delaying
    # the DMA issues and anchoring neuron-profile's first_useful_time.
    for bb in nc.main_func.blocks:
        dead = [
            i
            for i in bb.instructions
            if i.opcode == "Memset"
            and i.outs
            and str(getattr(i.outs[0], "memref", "")).startswith("const-")
        ]
        for i in dead:
            bb.instructions.remove(i)

    nc.compile()
    return nc


_NC_CACHE: list = []


def _get_nc() -> bass.Bass:
    if not _NC_CACHE:
        _NC_CACHE.append(build_nc())
    return _NC_CACHE[0]


def _make_in_maps(s, h, W, U, v):
    s2 = np.ascontiguousarray(np.asarray(s, np.float32).reshape(B * L, D))
    h2 = np.asarray(h, np.float32)
    W2 = np.asarray(W, np.float32)
    U2 = np.asarray(U, np.float32).astype(ml_dtypes.bfloat16)
    v2 = np.asarray(v, np.float32)
    vmat = np.zeros((D, G * G), np.float32)
    for j in range(G):
        vmat[:, j * G + j] = v2[:, 0]
    aux_tail = np.zeros((128, IDW + 2), np.float32)
    aux_tail[:IDW, :IDW] = np.eye(IDW, dtype=np.float32)
    in_maps = []
    for c in range(N_CORES):
        b = c * QPC // L
        h_b = h2[b]  # [L, D]
        hb = h_b.reshape(MT, 128, D).transpose(1, 0, 2).reshape(128, MT * D)
        aux = np.concatenate([vmat, hb, aux_tail], axis=1)
        in_maps.append(
            {
                "hTU": np.ascontiguousarray(
                    np.concatenate(
                        [h_b.T, U2.astype(np.float32)], axis=1
                    ).astype(ml_dtypes.bfloat16)
                ),
                "WsT": np.ascontiguousarray(
                    np.concatenate(
                        [W2, s2[c * QPC : (c + 1) * QPC].T], axis=1
                    )
                ),
                "aux": np.ascontiguousarray(aux.astype(ml_dtypes.bfloat16)),
            }
        )
    return in_maps


def run_spmd(s, h, W, U, v, **kwargs):
    """Run the kernel on 8 cores; returns the BassKernelResults."""
    nc = _get_nc()
    in_maps = _make_in_maps(s, h, W, U, v)
    return run_bass_kernel_spmd(nc, in_maps, core_ids=list(range(N_CORES)), **kwargs)


def kernel(s, h, W, U, v):
    res = run_spmd(s, h, W, U, v)
    shards = [np.asarray(res.results[c]["out"]) for c in range(N_CORES)]
    return np.concatenate(shards, axis=0).reshape(B, L, D).astype(np.float32)
